# revision 13
# baseline (speedup 1.0000x reference)
"""DCRNN (2-layer encoder/decoder DCGRU, N=512 nodes, B=32, U=64, K=2, 2 supports)
Trainium2 Bass/Tile kernel, data-parallel over batch across 8 NeuronCores.

Reformulation: gconv(X) = sum_m T_m(S) @ X @ W_m, T in {S_A, 2S_A^2-I, S_B,
2S_B^2-I} precomputed on host (identity term handled densely). Per gconv:
  stage 1 (dense):     A_m = X @ W_m      -- X^T chunks as lhsT, W as rhs ->
                       A node-on-partition
  stage 2 (diffusion): out^T = X@W_0 (dense) + sum_m (T_m A_m)^T -- A_m as
                       lhsT, T_m^T as rhs, accumulated in PSUM; bias +
                       sigmoid/tanh fused into the PSUM->SBUF move.
All matmul operands bf16 (PSUM accumulation fp32); activations/state bf16.
No tensor transposes at runtime; weights/supports/state resident in SBUF for
the whole 24-step recurrence.
"""

import sys

sys.path.insert(0, "/opt/trn_rl_repo")

import numpy as np

import concourse.bass as bass
import concourse.mybir as mybir
import concourse.tile as tile
from concourse import bacc, bass_utils

# Model dims (fixed by the problem)
N = 512
T_ENC = 12
HOR = 12
U = 64
NM = 4  # diffusion matrices [S_A, 2S_A^2, S_B, 2S_B^2]; identity folded into W_0
B = 32
NCORES = 8
BL = B // NCORES  # local batch = 4
BI = BL * N  # 2048: the (b, node) free dim
C0 = 1 + U  # 65 input channels, layer 0
C1 = U + U  # 128 input channels, layer 1
KCH = N // 128  # 4 node chunks

F32 = mybir.dt.float32
BF16 = mybir.dt.bfloat16
F8 = mybir.dt.float8e4
TSCALE = 16.0  # fp8 diffusion operand scale (undone in the activations)
AF = mybir.ActivationFunctionType

LDW_OPT = False  # enable walrus ldw-opt pass (default cmdline disables it)

_ldw_patched = False


def _patch_ldw_opt():
    global _ldw_patched
    if _ldw_patched or not LDW_OPT:
        return
    _ldw_patched = True
    orig = bass_utils.bir_verify_and_optimise

    def patched(tmpdir, inp="bir.json", outp="file.neff", arch=None, *, dve_root=None):
        import concourse.bass_utils as bu

        real_run = bu.run_command

        def run_hook(cmd, **kw):
            cmd = [
                c.replace("--enable-ldw-opt=false", "--enable-ldw-opt=true")
                for c in cmd
            ]
            return real_run(cmd, **kw)

        bu.run_command = run_hook
        try:
            return orig(tmpdir, inp, outp, arch, dve_root=dve_root)
        finally:
            bu.run_command = real_run

    bass_utils.bir_verify_and_optimise = patched


def _build_program(n_enc=T_ENC, n_dec=HOR):
    _patch_ldw_opt()
    nc = bacc.Bacc("TRN2", target_bir_lowering=False, debug=False)

    # ---- DRAM I/O ----
    d_xenc = nc.dram_tensor("xenc", [n_enc, BI], BF16, kind="ExternalInput")
    d_tmf8 = nc.dram_tensor("tmf8", [NM * 4 * 128, 512], F8, kind="ExternalInput")
    d_tmbf = nc.dram_tensor("tmbf", [NM * 4 * 128, 512], BF16, kind="ExternalInput")
    d_w = {}
    for pfx in ("e", "d"):
        for lyr, c_in in ((0, C0), (1, C1)):
            d_w[f"{pfx}wg{lyr}"] = nc.dram_tensor(
                f"{pfx}wg{lyr}", [c_in, 5 * 2 * U], BF16, kind="ExternalInput"
            )
            d_w[f"{pfx}wc{lyr}"] = nc.dram_tensor(
                f"{pfx}wc{lyr}", [c_in, 5 * U], BF16, kind="ExternalInput"
            )
            d_w[f"{pfx}bgr{lyr}"] = nc.dram_tensor(
                f"{pfx}bgr{lyr}", [U, 1], F32, kind="ExternalInput"
            )
            d_w[f"{pfx}bgu{lyr}"] = nc.dram_tensor(
                f"{pfx}bgu{lyr}", [U, 1], F32, kind="ExternalInput"
            )
            d_w[f"{pfx}bc{lyr}"] = nc.dram_tensor(
                f"{pfx}bc{lyr}", [U, 1], F32, kind="ExternalInput"
            )
    d_pw = nc.dram_tensor("pw", [U, 2], BF16, kind="ExternalInput")
    d_zeros = nc.dram_tensor("zeros", [C1, BI], BF16, kind="ExternalInput")
    d_pb = nc.dram_tensor("pb", [1, 1], F32, kind="ExternalInput")
    d_out = nc.dram_tensor("outs", [n_dec, BI], BF16, kind="ExternalOutput")

    with tile.TileContext(nc) as tc:
        _body(tc, n_enc, n_dec, d_xenc, d_tmf8, d_tmbf, d_w, d_pw, d_pb, d_zeros, d_out)
    nc.compile()
    return nc


def _body(tc, n_enc, n_dec, d_xenc, d_tmf8, d_tmbf, d_w, d_pw, d_pb, d_zeros, d_out):
    nc = tc.nc
    consts = tc.alloc_tile_pool(name="consts", bufs=1)
    work = tc.alloc_tile_pool(name="work", bufs=1)
    gpool = tc.alloc_tile_pool(name="gpool", bufs=2)
    ag_pool = tc.alloc_tile_pool(name="agp", bufs=12)
    ac_pool = tc.alloc_tile_pool(name="acp", bufs=6)
    ps1 = tc.alloc_tile_pool(name="ps1", bufs=4, space="PSUM")
    ps2 = tc.alloc_tile_pool(name="ps2", bufs=4, space="PSUM")

    # ---- resident constants ----
    tm_f8 = consts.tile([128, NM, 2, 2, 512], F8, name="tm_f8")
    tm_bf = consts.tile([128, NM, KCH, 512], BF16, name="tm_bf")
    for m in range(NM):
        for k in range(KCH):
            row = (m * KCH + k) * 128
            nc.sync.dma_start(out=tm_f8[:, m, k // 2, k % 2, :], in_=d_tmf8[row : row + 128, :])
            nc.sync.dma_start(out=tm_bf[:, m, k, :], in_=d_tmbf[row : row + 128, :])

    w_sb = {}
    for key, dt in d_w.items():
        shape = list(dt.shape)
        w_sb[key] = consts.tile(shape, dt.dtype, name=f"sb_{key}")
        nc.sync.dma_start(out=w_sb[key][:, :], in_=dt[:, :])
    pw_sb = consts.tile([U, 2], BF16, name="pw_sb")
    nc.sync.dma_start(out=pw_sb[:, :], in_=d_pw[:, :])
    pb_sb = consts.tile([1, 1], F32, name="pb_sb")
    nc.sync.dma_start(out=pb_sb, in_=d_pb[:, :])

    # ---- persistent state (channel-on-partition, free dim = (b, node)) ----
    X0 = work.tile([C0, BI], BF16, name="X0")  # [h0 ; x]
    X0c = work.tile([C0, BI], BF16, name="X0c")  # [r*h0 ; x]
    X1 = work.tile([C1, BI], BF16, name="X1")  # [h1 ; h0]
    X1c = work.tile([C1, BI], BF16, name="X1c")  # [r*h1 ; h0]

    nc.sync.dma_start(out=X0[0:U, :], in_=d_zeros[0:U, :])
    nc.sync.dma_start(out=X1[:, :], in_=d_zeros[:, :])

    # collapse all load/init dependencies into one semaphore so the first
    # consumers don't exceed per-instruction sync-wait slots
    tc.strict_bb_all_engine_barrier()

    def cell_phases(X, Xc, c_in, h_src, wg, bgr, bgu, wc, bc, h_writer, post):
        """One DCGRU cell, split into per-batch-pair phases so independent
        pairs' matmuls cover each other's activation/elementwise tails.
        Returns (gate_phase, cand_phase), each callable with p in {0, 1}."""
        R = gpool.tile([U, BI], BF16, tag="R", name="R", bufs=1)
        Uu = gpool.tile([U, BI], BF16, tag="Uu", name="Uu", bufs=1)
        Wu = gpool.tile([U, BI], BF16, tag="Wu", name="Wu", bufs=1)
        uh = gpool.tile([U, BI], BF16, tag="uh", name="uh", bufs=1)
        Ct = gpool.tile([U, BI], BF16, tag="Ct", name="Ct", bufs=1)
        wc_t = gpool.tile([U, BI], BF16, tag="wct", name="wc_t", bufs=1)
        ag = {}
        ac = {}

        def gate_phase(p):
            # stage 1: A_m = X @ Wg_m for m=1..4 (m=0 done densely in stage 2)
            for b in (2 * p, 2 * p + 1):
                for k in range(KCH):
                    pg = ps1.tile([128, 512], F32, tag="s1", name="pg")
                    lhsT = X[0:c_in, b * N + k * 128 : b * N + (k + 1) * 128]
                    nc.tensor.matmul(
                        pg, lhsT, wg[:, 128:640], start=True, stop=True
                    )
                    if k % 2 == 0:
                        ag[(b, k // 2)] = ag_pool.tile(
                            [128, 2, 4, 128], F8, tag="ag", name="ag"
                        )
                    dst = ag[(b, k // 2)][:, k % 2, :, :]
                    src_v = pg[:, :].rearrange("p (m c) -> p m c", m=4)
                    if k % 2 == 0:
                        nc.scalar.copy(out=dst, in_=src_v)
                    else:
                        nc.vector.tensor_copy(out=dst, in_=src_v)
            # stage 2: acc = X @ Wg_0 + sum_m (T_m A_m)^T, fused sigmoid
            for b in (2 * p, 2 * p + 1):
                acc = ps2.tile([128, 512], F32, tag="s2", name="accg")
                for m in range(NM):
                    for kp in range(2):
                        nc.tensor.matmul(
                            acc,
                            ag[(b, kp)][:, :, m, :],
                            tm_f8[:, m, kp, :, :],
                            start=(m == 0 and kp == 0),
                            stop=False,
                            perf_mode=mybir.MatmulPerfMode.DoubleRow,
                        )
                # dense m0 last: its rhs (X incl. the x row) arrives latest in
                # the decoder; keeping it off the queue head avoids PE stalls
                nc.tensor.matmul(
                    acc,
                    wg[:, 0:128],
                    X[0:c_in, b * N : (b + 1) * N],
                    start=False,
                    stop=True,
                )
                bcols = slice(b * N, (b + 1) * N)
                nc.scalar.activation(
                    out=R[:, bcols], in_=acc[0:U, :], func=AF.Sigmoid,
                    bias=bgr[:, 0:1], scale=1.0 / TSCALE,
                )
                nc.scalar.activation(
                    out=Uu[:, bcols], in_=acc[U : 2 * U, :], func=AF.Sigmoid,
                    bias=bgu[:, 0:1], scale=1.0 / TSCALE,
                )
            pcols = slice(2 * p * N, 2 * (p + 1) * N)
            # 1-u on the vector engine (frees scalar-engine time)
            nc.vector.tensor_scalar(
                out=Wu[:, pcols], in0=Uu[:, pcols], scalar1=-1.0, scalar2=1.0,
                op0=mybir.AluOpType.mult, op1=mybir.AluOpType.add,
            )
            # r*h -> candidate input rows; u*h for the GRU blend
            nc.vector.tensor_mul(
                out=Xc[0:U, pcols],
                in0=R[:, pcols],
                in1=h_src[:, pcols],
            )
            nc.vector.tensor_mul(
                out=uh[:, pcols],
                in0=Uu[:, pcols],
                in1=h_src[:, pcols],
            )

        def cand_phase(p):
            # stage 1: A_m = Xc @ Wc_m for m=1..4; two batches packed into the
            # 128 output-channel partitions of stage 2
            for b in (2 * p, 2 * p + 1):
                for k in range(KCH):
                    pc = ps1.tile([128, 512], F32, tag="s1", name="pc")
                    lhsT = Xc[0:c_in, b * N + k * 128 : b * N + (k + 1) * 128]
                    nc.tensor.matmul(
                        pc[:, 0:256], lhsT, wc[:, U : 5 * U], start=True, stop=True
                    )
                    if b % 2 == 0:
                        ac[(p, k)] = ac_pool.tile(
                            [128, NM, 2, U], BF16, tag="ac", name="ac"
                        )
                    dst = ac[(p, k)][:, :, b % 2, :]
                    src_v = pc[:, 0:256].rearrange("p (m u) -> p m u", m=NM)
                    if k % 2 == 0:
                        nc.scalar.copy(out=dst, in_=src_v)
                    else:
                        nc.vector.tensor_copy(out=dst, in_=src_v)
            # stage 2: bf16 diffusion over m=1..4, then the dense folded m0
            # into the two batch halves (its Xc rhs arrives latest)
            acc = ps2.tile([128, 512], F32, tag="s2", name="accc")
            for m in range(NM):
                for k in range(KCH):
                    nc.tensor.matmul(
                        acc,
                        ac[(p, k)][:, m, :, :],
                        tm_bf[:, m, k, :],
                        start=(m == 0 and k == 0),
                        stop=False,
                    )
            nc.tensor.matmul(
                acc[0:U, :],
                wc[:, 0:U],
                Xc[0:c_in, 2 * p * N : (2 * p + 1) * N],
                start=False,
                stop=False,
                tile_position=(0, 0),
            )
            nc.tensor.matmul(
                acc[U : 2 * U, :],
                wc[:, 0:U],
                Xc[0:c_in, (2 * p + 1) * N : (2 * p + 2) * N],
                start=False,
                stop=True,
                tile_position=(0, 64),
            )
            for half in range(2):
                b = 2 * p + half
                bcols = slice(b * N, (b + 1) * N)
                nc.scalar.activation(
                    out=Ct[:, bcols],
                    in_=acc[half * U : (half + 1) * U, :],
                    func=AF.Tanh,
                    bias=bc[:, 0:1],
                    scale=1.0,
                )
            pcols = slice(2 * p * N, 2 * (p + 1) * N)
            # h_new = u*h + (1-u)*c
            nc.vector.tensor_mul(out=wc_t[:, pcols], in0=Wu[:, pcols], in1=Ct[:, pcols])
            nc.vector.tensor_add(
                out=h_writer(p, pcols), in0=uh[:, pcols], in1=wc_t[:, pcols]
            )
            post(p, pcols)

        return gate_phase, cand_phase

    def l0_writer(p, pcols):
        return X0[0:U, pcols]

    def l0_post(p, pcols):
        # propagate h0 into the layer-1 input tiles ([h1 ; h0] / [r*h1 ; h0])
        nc.scalar.copy(out=X1[U:C1, pcols], in_=X0[0:U, pcols])
        nc.vector.tensor_copy(out=X1c[U:C1, pcols], in_=X0[0:U, pcols])

    def l1_writer(p, pcols):
        return X1[0:U, pcols]

    def l1_post(p, pcols):
        pass

    def proj_phase(p):
        # projection for pair p: out = h1 . pw + pb -> feeds back as x row
        for q in (2 * p, 2 * p + 1):
            pp = ps2.tile([2, 512], F32, tag="s2", name="pp")
            nc.tensor.matmul(
                pp,
                pw_sb[:, 0:2],
                X1[0:U, q * 512 : (q + 1) * 512],
                start=True,
                stop=True,
            )
            nc.scalar.activation(
                out=X0[U:C0, q * 512 : (q + 1) * 512],
                in_=pp[0:1, :],
                func=AF.Identity,
                bias=pb_sb[:, 0:1],
                scale=1.0,
            )
            nc.vector.tensor_scalar_add(
                out=X0c[U:C0, q * 512 : (q + 1) * 512], in0=pp[0:1, :],
                scalar1=pb_sb[0:1, 0:1],
            )

    def run_step(pfx, dec_t=None):
        g0, c0 = cell_phases(
            X0, X0c, C0, X0[0:U, :],
            w_sb[f"{pfx}wg0"], w_sb[f"{pfx}bgr0"], w_sb[f"{pfx}bgu0"],
            w_sb[f"{pfx}wc0"], w_sb[f"{pfx}bc0"],
            l0_writer, l0_post,
        )
        g1, c1 = cell_phases(
            X1, X1c, C1, X1[0:U, :],
            w_sb[f"{pfx}wg1"], w_sb[f"{pfx}bgr1"], w_sb[f"{pfx}bgu1"],
            w_sb[f"{pfx}wc1"], w_sb[f"{pfx}bc1"],
            l1_writer, l1_post,
        )
        g0(0); g0(1); c0(0); c0(1)
        g1(0); g1(1); c1(0)
        if dec_t is None:
            c1(1)
        else:
            proj_phase(0)
            c1(1)
            proj_phase(1)
            nc.sync.dma_start(out=d_out[dec_t : dec_t + 1, :], in_=X0[U:C0, :])

    # ================= encoder =================
    for t in range(n_enc):
        nc.sync.dma_start(out=X0[U:C0, :], in_=d_xenc[t : t + 1, :])
        nc.sync.dma_start(out=X0c[U:C0, :], in_=d_xenc[t : t + 1, :])
        run_step("e")

    # ================= decoder =================
    nc.sync.dma_start(out=X0[U:C0, :], in_=d_zeros[0:1, :])
    nc.sync.dma_start(out=X0c[U:C0, :], in_=d_zeros[0:1, :])
    for t in range(n_dec):
        run_step("d", dec_t=t)

    for pool in (ps2, ps1, ac_pool, ag_pool, gpool, work, consts):
        pool.release()


# --------------------------------------------------------------------------
# host-side packing
# --------------------------------------------------------------------------
def _bf16(a):
    import ml_dtypes

    return np.ascontiguousarray(np.asarray(a, np.float32).astype(ml_dtypes.bfloat16))


def _prep_shared(inputs):
    sup = np.asarray(inputs["supports"], np.float64)
    tms = [
        sup[0],
        2.0 * (sup[0] @ sup[0]),
        sup[1],
        2.0 * (sup[1] @ sup[1]),
    ]
    f8 = mybir.dt.np(F8)
    tmats = np.stack([t.T for t in tms]).astype(np.float32)  # [m, j, i]
    tmats = tmats.reshape(NM * 4 * 128, 512)

    shared = {
        "tmf8": np.ascontiguousarray((tmats * TSCALE).astype(f8)),
        "tmbf": _bf16(tmats),
    }
    for pfx, name in (("e", "enc"), ("d", "dec")):
        for lyr, c_in in ((0, C0), (1, C1)):
            wg = np.asarray(inputs[f"{name}{lyr}_Wg"], np.float32)
            wc = np.asarray(inputs[f"{name}{lyr}_Wc"], np.float32)
            bg = np.asarray(inputs[f"{name}{lyr}_bg"], np.float32)
            bc = np.asarray(inputs[f"{name}{lyr}_bc"], np.float32)
            # rows are (c, m) c-major; regroup columns m-major
            wg_r = wg.reshape(c_in, 5, 2 * U).reshape(c_in, 5 * 2 * U)
            wc_r = wc.reshape(c_in, 5, U).reshape(c_in, 5 * U)
            if lyr == 0:
                perm = np.r_[1:c_in, 0]  # X0 rows [h0 ; x]
            else:
                perm = np.r_[U:c_in, 0:U]  # X1 rows [h1 ; h0]
            wg_r = wg_r[perm].copy()
            wc_r = wc_r[perm].copy()
            # fold the -I part of 2S^2-I into the m0 weight block
            wg_r[:, 0:128] -= wg_r[:, 256:384] + wg_r[:, 512:640]
            wc_r[:, 0:U] -= wc_r[:, 2 * U : 3 * U] + wc_r[:, 4 * U : 5 * U]
            wg_r[:, 0:128] *= TSCALE  # match the fp8 diffusion scale
            shared[f"{pfx}wg{lyr}"] = _bf16(wg_r)
            shared[f"{pfx}wc{lyr}"] = _bf16(wc_r)
            shared[f"{pfx}bgr{lyr}"] = np.ascontiguousarray(bg[:U].reshape(U, 1))
            shared[f"{pfx}bgu{lyr}"] = np.ascontiguousarray(bg[U:].reshape(U, 1))
            shared[f"{pfx}bc{lyr}"] = np.ascontiguousarray(bc.reshape(U, 1))
    pw = np.asarray(inputs["proj_W"], np.float32).reshape(U, 1)
    shared["pw"] = _bf16(np.concatenate([pw, np.zeros((U, 1), np.float32)], axis=1))
    shared["pb"] = np.asarray(inputs["proj_b"], np.float32).reshape(1, 1)
    shared["zeros"] = _bf16(np.zeros((C1, BI), np.float32))
    return shared


def _make_in_maps(inputs, n_enc=T_ENC):
    shared = _prep_shared(inputs)
    x = np.asarray(inputs["inputs"], np.float32)  # (T, B, N)
    in_maps = []
    for c in range(NCORES):
        m = dict(shared)
        m["xenc"] = _bf16(x[:n_enc, c * BL : (c + 1) * BL, :].reshape(n_enc, BI))
        in_maps.append(m)
    return in_maps


_PROG_CACHE = {}


def _get_program(n_enc=T_ENC, n_dec=HOR):
    key = (n_enc, n_dec)
    if key not in _PROG_CACHE:
        _PROG_CACHE[key] = _build_program(n_enc, n_dec)
    return _PROG_CACHE[key]


def _run(inputs, n_enc=T_ENC, n_dec=HOR, **kw):
    nc = _get_program(n_enc, n_dec)
    in_maps = _make_in_maps(inputs, n_enc)
    res = bass_utils.run_bass_kernel_spmd(nc, in_maps, core_ids=list(range(NCORES)), **kw)
    out = np.empty((n_dec, B, N), np.float32)
    for c in range(NCORES):
        out[:, c * BL : (c + 1) * BL, :] = (
            np.asarray(res.results[c]["outs"]).astype(np.float32).reshape(n_dec, BL, N)
        )
    return out.reshape(n_dec, B, N), res


def kernel(**inputs) -> np.ndarray:
    out, _ = _run(inputs)
    return out.reshape(HOR, B, N)
